# revision 8
# baseline (speedup 1.0000x reference)
"""HEPT sparse attention for Trainium2 — 8-core SPMD Bass kernel.

Reference computation (per hash-round r, head h):
  hash q/k via shared projection, argsort, gather into blocks of 128,
  blocked RBF attention: so = exp(-0.5*||q_i-k_j||^2) @ v.

Strategy:
  - Host: bitwise-exact hash + argsort (jax CPU, identical ops to the
    reference), gather, fp16 quantization, and layout packing.
  - Device (per core, 3 of the 24 (r,h) units = 768 blocks): the whole
    -0.5*||q-k||^2 Gram matrix of one block is ONE K=32 fp16 matmul:
    K rows 0-26 carry q/k^T (fp16), rows 27-30 carry the squared-norm
    terms (-0.5*||.||^2 split hi/lo fp16, paired with ones rows), row 31
    is zero padding. Then exp on ACT (batched over 8 blocks of PSUM),
    then so = A @ v as a second matmul (A fp16 weights, v fp16 moving).
  - 4 blocks share the 128-partition dim via PE row-tiling
    (tile_position=(32*lane, 0)), so mm1 of 4 blocks runs concurrently.

Numerics (validated on host): global rel err ~9.3e-4 vs reference.
"""

import os
from contextlib import ExitStack

import numpy as np

# ---- problem constants (hardcoded; kernel.py must be self-contained) ----
N_HASHES = 3
N_HEADS = 8
PADDED_SIZE = 32768
BLOCK = 128
DIM_PER_HEAD = 24
D_QK = 27
NB = PADDED_SIZE // BLOCK          # 256 blocks per (r,h)
N_CORES = 8
UNITS = N_HASHES * N_HEADS         # 24 independent (r,h) units
UPC = UNITS // N_CORES             # 3 units per core
NBLK = UPC * NB                    # 768 blocks per core
NQUAD = NBLK // 4                  # 192 quads per core (4 blocks/quad)
SUPER_Q = 16                       # quads per super-tile (64 blocks)
KROWS = 32                         # stacked contraction rows per block

_NC_CACHE = {}


# columns per super-tile in the combined input tensor (fp16):
#   q-stacks SUPER_Q*128 | k-stacks SUPER_Q*128 | v SUPER_Q*4*24
SUP_Q_COLS = SUPER_Q * 128
SUP_V_COLS = SUPER_Q * 4 * 24
SUP_COLS = 2 * SUP_Q_COLS + SUP_V_COLS


def build_nc(n_quads=NQUAD):
    """Build the per-core Bass module (same NEFF for all 8 cores).

    Constraints baked in (found the hard way):
    - One combined input DMA per super-tile (q|k|v): every PE/ACT
      instruction stays at <=2 distinct sem waits.
    - Row-tiled (tile_position) matmuls sharing a PSUM *bank* crash the
      device -> lane l writes bank l of a 4-bank ps1 tile.
    - mm2 reuses ps1 bank 0 after the exp ACTIVATE consumed ps1, so
      ps1 can double-buffer within the 8 PSUM banks.
    """
    import concourse.mybir as mybir
    import concourse.tile as tile
    from concourse.bacc import Bacc

    f16 = mybir.dt.float16
    f32 = mybir.dt.float32
    nblk = n_quads * 4
    assert n_quads % SUPER_Q == 0
    n_super = n_quads // SUPER_Q

    nc = Bacc()
    im = nc.declare_dram_parameter("in", [128, n_super * SUP_COLS], f16,
                                   isOutput=False)
    om = nc.declare_dram_parameter("out", [128, nblk * 24], f32, isOutput=True)

    with tile.TileContext(nc) as tc, ExitStack() as ctx:
        ipool = ctx.enter_context(tc.tile_pool(name="ipool", bufs=2))
        apool = ctx.enter_context(tc.tile_pool(name="apool", bufs=3))
        opool = ctx.enter_context(tc.tile_pool(name="opool", bufs=3))
        ps1pool = ctx.enter_context(tc.tile_pool(name="ps1", bufs=2, space="PSUM"))

        for s in range(n_super):
            in_t = ipool.tile([128, SUP_COLS], f16)
            c0 = s * SUP_COLS
            nc.sync.dma_start(out=in_t, in_=im[:, c0:c0 + SUP_COLS])
            q_t = in_t[:, 0:SUP_Q_COLS]
            k_t = in_t[:, SUP_Q_COLS:2 * SUP_Q_COLS]
            v_t = in_t[:, 2 * SUP_Q_COLS:SUP_COLS]

            for og in range(4):                 # 4 groups of 4 quads (16 blk)
                ps1 = ps1pool.tile([128, 2048], f32)
                for gq in range(4):
                    qd = og * 4 + gq            # quad idx within super
                    for l in range(4):          # 4 blocks per quad (lanes)
                        col = l * 512 + gq * 128
                        nc.tensor.matmul(
                            out=ps1[:, col:col + 128],
                            lhsT=k_t[32 * l:32 * l + 32, qd * 128:(qd + 1) * 128],
                            rhs=q_t[32 * l:32 * l + 32, qd * 128:(qd + 1) * 128],
                            start=True, stop=True,
                            tile_position=(32 * l, 0),
                        )
                a_t = apool.tile([128, 2048], f16)
                nc.scalar.activation(
                    out=a_t, in_=ps1,
                    func=mybir.ActivationFunctionType.Exp,
                )
                for bb in range(16):            # so = A @ v per block
                    gq, l = bb // 4, bb % 4
                    b_local = (og * 4 + gq) * 4 + l
                    acol = l * 512 + gq * 128
                    nc.tensor.matmul(
                        out=ps1[:, bb * 24:(bb + 1) * 24],
                        lhsT=a_t[:, acol:acol + 128],
                        rhs=v_t[:, b_local * 24:(b_local + 1) * 24],
                        start=True, stop=True,
                    )
                o_t = opool.tile([128, 16 * 24], f32)
                nc.vector.tensor_copy(out=o_t, in_=ps1[:, 0:384])
                b0 = (s * 4 + og) * 16
                nc.sync.dma_start(out=om[:, b0 * 24:(b0 + 16) * 24], in_=o_t)
    nc.finalize()
    return nc


def _get_nc(n_quads=NQUAD):
    if n_quads not in _NC_CACHE:
        _NC_CACHE[n_quads] = build_nc(n_quads)
    return _NC_CACHE[n_quads]


# ---------------- host-side preparation ----------------

def _sort_indices(query, key, combined_shifts, alpha):
    """Replicate the reference's hash + argsort with jax on CPU.

    Uses the exact same jnp ops the reference uses so the fp32 values
    (and therefore the argsort permutations) match bit-for-bit.
    """
    import jax
    import jax.numpy as jnp

    cpu = jax.devices("cpu")[0]
    with jax.default_device(cpu):
        q = jnp.asarray(query)
        k = jnp.asarray(key)
        al = jnp.asarray(alpha)
        cs_i = jnp.asarray(combined_shifts)
        q_hashed = jnp.einsum('hnd,hdr->rhn', q, al)
        k_hashed = jnp.einsum('hnd,hdr->rhn', k, al)
        max_shift = jnp.maximum(q_hashed.max(-1, keepdims=True),
                                k_hashed.max(-1, keepdims=True))
        min_shift = jnp.minimum(q_hashed.min(-1, keepdims=True),
                                k_hashed.min(-1, keepdims=True))
        hash_shift = max_shift - min_shift
        cs = cs_i.astype(q_hashed.dtype) * hash_shift
        q_pos = np.asarray(jnp.argsort(q_hashed + cs, axis=-1))
        k_pos = np.asarray(jnp.argsort(k_hashed + cs, axis=-1))
    return q_pos, k_pos


def _split16(x):
    hi = x.astype(np.float16)
    lo = (x - hi.astype(np.float32)).astype(np.float16)
    return hi, lo


def _build_stack(s_qk, is_k):
    """(UNITS*NB, 128, 27) f32 -> (UNITS*NB, 32, 128) fp16 stack."""
    nblk = s_qk.shape[0]
    hi = s_qk.astype(np.float16)                       # (b, i, d)
    sqm = -0.5 * np.einsum('bid,bid->bi', hi.astype(np.float32),
                           hi.astype(np.float32))      # (b, i) f32
    sq_hi, sq_lo = _split16(sqm)
    st = np.zeros((nblk, KROWS, BLOCK), np.float16)
    st[:, :D_QK, :] = hi.transpose(0, 2, 1)            # rows 0-26: x^T
    if is_k:
        st[:, 27, :] = 1.0                             # pair of q's sq rows
        st[:, 28, :] = 1.0
        st[:, 29, :] = sq_hi                           # -0.5*||k||^2 hi
        st[:, 30, :] = sq_lo
    else:
        st[:, 27, :] = sq_hi                           # -0.5*||q||^2 hi
        st[:, 28, :] = sq_lo
        st[:, 29, :] = 1.0                             # pair of k's sq rows
        st[:, 30, :] = 1.0
    return st


def _pack_core(stack_blocks):
    """(768, 32, 128) -> (128, 192*128): partition = lane*32+row,
    free = quad*128 + col."""
    return (stack_blocks.reshape(NQUAD, 4, KROWS, BLOCK)
            .transpose(1, 2, 0, 3)
            .reshape(128, NQUAD * BLOCK))


def prepare_in_maps(query, key, value, combined_shifts, alpha):
    query = np.ascontiguousarray(np.asarray(query), dtype=np.float32)
    key = np.ascontiguousarray(np.asarray(key), dtype=np.float32)
    value = np.ascontiguousarray(np.asarray(value), dtype=np.float32)
    combined_shifts = np.asarray(combined_shifts)
    alpha = np.asarray(alpha, dtype=np.float32)

    q_pos, k_pos = _sort_indices(query, key, combined_shifts, alpha)

    h_idx = np.arange(N_HEADS)[None, :, None]
    s_query = query[h_idx, q_pos].reshape(UNITS * NB, BLOCK, D_QK)
    s_key = key[h_idx, k_pos].reshape(UNITS * NB, BLOCK, D_QK)
    s_value = value[h_idx, k_pos].reshape(UNITS * NB, BLOCK, DIM_PER_HEAD)

    qstack = _build_stack(s_query, is_k=False)
    kstack = _build_stack(s_key, is_k=True)
    v16 = s_value.astype(np.float16)

    n_super = NQUAD // SUPER_Q
    in_maps = []
    for c in range(N_CORES):
        b0, b1 = c * NBLK, (c + 1) * NBLK
        qp = _pack_core(qstack[b0:b1])              # [128, NQUAD*128]
        kp = _pack_core(kstack[b0:b1])
        vp = v16[b0:b1].transpose(1, 0, 2).reshape(128, NBLK * 24)
        combined = np.empty((128, n_super * SUP_COLS), np.float16)
        for s in range(n_super):
            c0 = s * SUP_COLS
            combined[:, c0:c0 + SUP_Q_COLS] = \
                qp[:, s * SUP_Q_COLS:(s + 1) * SUP_Q_COLS]
            combined[:, c0 + SUP_Q_COLS:c0 + 2 * SUP_Q_COLS] = \
                kp[:, s * SUP_Q_COLS:(s + 1) * SUP_Q_COLS]
            combined[:, c0 + 2 * SUP_Q_COLS:c0 + SUP_COLS] = \
                vp[:, s * SUP_V_COLS:(s + 1) * SUP_V_COLS]
        in_maps.append({"in": combined})
    return in_maps


def assemble_output(results):
    """results: list of 8 dicts with 'out' [128, 768*24] f32."""
    out = np.empty((UNITS, NB, BLOCK, DIM_PER_HEAD), np.float32)
    for c in range(N_CORES):
        so = np.asarray(results[c]["out"]).reshape(128, NBLK, 24)
        out[c * UPC:(c + 1) * UPC] = (
            so.transpose(1, 0, 2).reshape(UPC, NB, BLOCK, DIM_PER_HEAD))
    return out.reshape(N_HASHES, N_HEADS, NB, BLOCK, DIM_PER_HEAD)


def run(query, key, value, combined_shifts, alpha, trace=False):
    from concourse.bass_utils import run_bass_kernel_spmd

    in_maps = prepare_in_maps(query, key, value, combined_shifts, alpha)
    nc = _get_nc()
    res = run_bass_kernel_spmd(
        nc, in_maps, core_ids=list(range(N_CORES)), trace=trace)
    out = assemble_output(res.results)
    return out, res


def kernel(query, key, value, combined_shifts, alpha):
    out, _ = run(query, key, value, combined_shifts, alpha,
                 trace=bool(int(os.environ.get("HEPT_TRACE", "0"))))
    return out
